# revision 7
# baseline (speedup 1.0000x reference)
"""Grouped per-channel Linear + ReLU on 8 TRN2 NeuronCores.

Problem: out[b,c,e] = relu(sum_s x[b,s,c] * W[c,s,e] + bias[c,e])
  x: (256, 2048, 32) f32, W: (32, 2048, 2048) f32, bias: (32, 2048) f32
  out: (256, 32, 2048) f32

Sharding: expert/channel parallel — core i computes channels [4i, 4i+4).
Each core runs 4 independent GEMMs of (256x2048)@(2048x2048) with the
contraction dim S on SBUF partitions. x is pre-transposed on the host to
(C, S, B) so both matmul operands stream from DRAM with contiguous rows.

Per channel: x slab (S,B) loaded once (1 MB DMA); W streamed in 2 MB
chunks (4 k-tiles x full E row) for DMA efficiency; all 8 PSUM banks hold
the (2 bt x 4 et) output block accumulating over 16 k-tiles. The bias is
folded in as an extra K=1 matmul (lhsT = ones(1,128), rhs = bias row).
ReLU happens during PSUM->SBUF eviction on the Scalar engine, and each
(bt, c) output row goes out as one 1 MB DMA.
"""

import os
import sys

for _p in ("/opt/trn_rl_repo", "/root/.axon_site/_ro/trn_rl_repo"):
    if os.path.isdir(_p) and _p not in sys.path:
        sys.path.insert(0, _p)

import numpy as np
import ml_dtypes

import concourse.bacc as bacc
import concourse.mybir as mybir
from concourse import tile
from concourse.bass_utils import run_bass_kernel_spmd

B, S, C, E = 256, 2048, 32, 2048
NCORES = 8
CPC = C // NCORES          # channels per core = 4
P = 128
KT = S // P                # 16 k-tiles
NBT = B // P               # 2 batch tiles
FREE = 512                 # matmul moving free dim (one PSUM bank of f32)
NET = E // FREE            # 4 e-tiles
KC = 4                     # k-tiles per W DMA chunk (2 MB chunks)

# matmul dtype: "bfloat16" (fast, ~2e-3 rel), "float32r" (~1.5e-4 rel,
# DMA-bound 2x slower), "float32" (exact, 4x slower compute-bound)
MM_DTYPE = os.environ.get("KERNEL_MM_DTYPE", "bfloat16")

_DTYPES = {
    "bfloat16": (mybir.dt.bfloat16, ml_dtypes.bfloat16),
    "float32r": (mybir.dt.float32r, np.float32),
    "float32": (mybir.dt.float32, np.float32),
}

_nc_cache = {}


def _build(mm_dtype: str):
    mm_dt, _ = _DTYPES[mm_dtype]
    nc = bacc.Bacc(None, target_bir_lowering=False)
    xt = nc.dram_tensor("xt", [CPC, S, B], mm_dt, kind="ExternalInput")
    w = nc.dram_tensor("w", [CPC, S, E], mm_dt, kind="ExternalInput")
    bias = nc.dram_tensor("bias", [CPC, E], mm_dt, kind="ExternalInput")
    out = nc.dram_tensor("out", [B, CPC, E], mybir.dt.float32, kind="ExternalOutput")

    with tile.TileContext(nc) as tc:
        XP = KT // KC  # x pieces per channel (4, 256 KB each)
        with (
            tc.tile_pool(name="const", bufs=1) as const,
            tc.tile_pool(name="xpool", bufs=2 * XP) as xpool,
            tc.tile_pool(name="bpool", bufs=2) as bpool,
            tc.tile_pool(name="wpool", bufs=4) as wpool,
            tc.tile_pool(name="opool", bufs=3) as opool,
            tc.tile_pool(name="psum", bufs=NBT * NET, space="PSUM") as psum,
        ):
            ones = const.tile([1, P], mm_dt)
            nc.any.memset(ones[:], 1.0)
            zbias = const.tile([P, 1], mybir.dt.float32)
            nc.any.memset(zbias[:], 0.0)

            def x_piece_dma(eng, xp, c, kc):
                eng.dma_start(
                    xp[:],
                    xt[c, kc * KC * P : (kc + 1) * KC * P, :].rearrange(
                        "(k p) b -> p k b", p=P
                    ),
                )

            for c in range(CPC):
                # x for this channel, in 4 pieces of [P, KC, B].
                # c == 0 is latency-critical: pieces go on the SP-HWDGE (sync)
                # ring interleaved ahead of the W chunks they feed. Later
                # channels prefetch on the SWDGE (gpsimd) ring, which doesn't
                # queue behind the W stream.
                xps = [
                    xpool.tile([P, KC, B], mm_dt, name="xp") for _ in range(XP)
                ]
                if c == 0:
                    x_piece_dma(nc.sync, xps[0], c, 0)
                else:
                    for kc in range(XP):
                        x_piece_dma(nc.gpsimd, xps[kc], c, kc)
                bsb = bpool.tile([1, E], mm_dt)
                nc.gpsimd.dma_start(bsb[:], bias[c : c + 1, :])

                ps = [
                    [
                        psum.tile([P, FREE], mybir.dt.float32, name="ps")
                        for _ in range(NET)
                    ]
                    for _ in range(NBT)
                ]
                # W chunk schedule (k-tiles per DMA): ramp up at kernel start
                # so the first matmuls don't wait on a full 2 MB transfer.
                chunk_kts = [1, 1, 2, 4, 4, 4] if c == 0 else [KC] * (KT // KC)
                k = 0
                for ckt in chunk_kts:
                    if c == 0:
                        # emit the x piece feeding k+ckt.. just ahead of its W
                        nxt = (k + ckt) // KC
                        if nxt > k // KC and nxt < XP:
                            x_piece_dma(nc.sync, xps[nxt], c, nxt)
                    wsb = wpool.tile([P, KC, E], mm_dt, name="wsb")
                    nc.sync.dma_start(
                        wsb[:, :ckt, :],
                        w[c, k * P : (k + ckt) * P, :].rearrange(
                            "(k p) e -> p k e", p=P
                        ),
                    )
                    for kk in range(ckt):
                        for bt in range(NBT):
                            lhsT = xps[k // KC][:, k % KC, bt * P : (bt + 1) * P]
                            for et in range(NET):
                                nc.tensor.matmul(
                                    ps[bt][et][:],
                                    lhsT,
                                    wsb[:, kk, et * FREE : (et + 1) * FREE],
                                    start=(k == 0),
                                    stop=False,
                                )
                        k += 1
                # bias row: psum += ones(1,128).T @ bias(1,FREE)
                for bt in range(NBT):
                    for et in range(NET):
                        nc.tensor.matmul(
                            ps[bt][et][:],
                            ones[0:1, :],
                            bsb[0:1, et * FREE : (et + 1) * FREE],
                            start=False,
                            stop=True,
                        )
                # Evict with fused ReLU, split across ScalarE (activation) and
                # VectorE (max with 0) so PSUM banks free up ~2x faster.
                last = c == CPC - 1
                for bt in range(NBT):
                    ot = opool.tile([P, E], mybir.dt.float32)
                    for et in range(NET):
                        dst = ot[:, et * FREE : (et + 1) * FREE]
                        if et % 2 == 0:
                            nc.scalar.activation(
                                dst,
                                ps[bt][et][:],
                                mybir.ActivationFunctionType.Relu,
                                bias=zbias[:],
                            )
                        else:
                            nc.vector.tensor_scalar_max(dst, ps[bt][et][:], 0.0)
                        if last:
                            # tail: small per-et DMAs so the final writes
                            # start as soon as each subtile is ready
                            nc.scalar.dma_start(
                                out[
                                    bt * P : (bt + 1) * P,
                                    c,
                                    et * FREE : (et + 1) * FREE,
                                ],
                                dst,
                            )
                    if not last:
                        # one 1 MB DMA per (bt, c) on the ACT HWDGE ring,
                        # separate from the W stream
                        nc.scalar.dma_start(out[bt * P : (bt + 1) * P, c, :], ot[:])
    nc.compile()
    return nc


def _get_nc(mm_dtype: str):
    if mm_dtype not in _nc_cache:
        _nc_cache[mm_dtype] = _build(mm_dtype)
    return _nc_cache[mm_dtype]


def _run(x, W, b, mm_dtype=None, **spmd_kwargs):
    mm_dtype = mm_dtype or MM_DTYPE
    _, np_dt = _DTYPES[mm_dtype]
    nc = _get_nc(mm_dtype)

    in_maps = []
    for i in range(NCORES):
        c0, c1 = i * CPC, (i + 1) * CPC
        xt_i = np.ascontiguousarray(
            x[:, :, c0:c1].transpose(2, 1, 0).astype(np_dt)
        )
        w_i = np.ascontiguousarray(W[c0:c1].astype(np_dt))
        b_i = np.ascontiguousarray(b[c0:c1].astype(np_dt))
        in_maps.append({"xt": xt_i, "w": w_i, "bias": b_i})

    res = run_bass_kernel_spmd(nc, in_maps, core_ids=list(range(NCORES)), **spmd_kwargs)
    out = np.concatenate([r["out"] for r in res.results], axis=1)
    return out, res


def kernel(x: np.ndarray, W: np.ndarray, b: np.ndarray) -> np.ndarray:
    out, _ = _run(x, W, b)
    return out


# revision 9
# speedup vs baseline: 1.0236x; 1.0236x over previous
"""Grouped per-channel Linear + ReLU on 8 TRN2 NeuronCores.

Problem: out[b,c,e] = relu(sum_s x[b,s,c] * W[c,s,e] + bias[c,e])
  x: (256, 2048, 32) f32, W: (32, 2048, 2048) f32, bias: (32, 2048) f32
  out: (256, 32, 2048) f32

Sharding: expert/channel parallel — core i computes channels [4i, 4i+4).
Each core runs 4 independent GEMMs of (256x2048)@(2048x2048) with the
contraction dim S on SBUF partitions. x is pre-transposed on the host to
(C, S, B) so both matmul operands stream from DRAM with contiguous rows.

Per channel: x slab (S,B) loaded once (1 MB DMA); W streamed in 2 MB
chunks (4 k-tiles x full E row) for DMA efficiency; all 8 PSUM banks hold
the (2 bt x 4 et) output block accumulating over 16 k-tiles. The bias is
folded in as an extra K=1 matmul (lhsT = ones(1,128), rhs = bias row).
ReLU happens during PSUM->SBUF eviction on the Scalar engine, and each
(bt, c) output row goes out as one 1 MB DMA.
"""

import os
import sys

for _p in ("/opt/trn_rl_repo", "/root/.axon_site/_ro/trn_rl_repo"):
    if os.path.isdir(_p) and _p not in sys.path:
        sys.path.insert(0, _p)

import numpy as np
import ml_dtypes

import concourse.bacc as bacc
import concourse.mybir as mybir
from concourse import tile
from concourse.bass_utils import run_bass_kernel_spmd

B, S, C, E = 256, 2048, 32, 2048
NCORES = 8
CPC = C // NCORES          # channels per core = 4
P = 128
KT = S // P                # 16 k-tiles
NBT = B // P               # 2 batch tiles
FREE = 512                 # matmul moving free dim (one PSUM bank of f32)
NET = E // FREE            # 4 e-tiles
KC = 4                     # k-tiles per W DMA chunk (2 MB chunks)

# matmul dtype: "bfloat16" (fast, ~2e-3 rel), "float32r" (~1.5e-4 rel,
# DMA-bound 2x slower), "float32" (exact, 4x slower compute-bound)
MM_DTYPE = os.environ.get("KERNEL_MM_DTYPE", "bfloat16")

_DTYPES = {
    "bfloat16": (mybir.dt.bfloat16, ml_dtypes.bfloat16),
    "float32r": (mybir.dt.float32r, np.float32),
    "float32": (mybir.dt.float32, np.float32),
}

_nc_cache = {}


def _build(mm_dtype: str):
    mm_dt, _ = _DTYPES[mm_dtype]
    nc = bacc.Bacc(None, target_bir_lowering=False)
    xt = nc.dram_tensor("xt", [CPC, S, B], mm_dt, kind="ExternalInput")
    w = nc.dram_tensor("w", [CPC, S, E], mm_dt, kind="ExternalInput")
    bias = nc.dram_tensor("bias", [CPC, E], mm_dt, kind="ExternalInput")
    out = nc.dram_tensor("out", [B, CPC, E], mybir.dt.float32, kind="ExternalOutput")

    with tile.TileContext(nc) as tc:
        XKC = 8        # k-tiles per x piece (2 pieces of 512 KB per channel)
        XP = KT // XKC
        with (
            tc.tile_pool(name="const", bufs=1) as const,
            tc.tile_pool(name="xpool", bufs=2 * XP) as xpool,
            tc.tile_pool(name="bpool", bufs=2) as bpool,
            tc.tile_pool(name="wpool", bufs=4) as wpool,
            tc.tile_pool(name="opool", bufs=3) as opool,
            tc.tile_pool(name="psum", bufs=NBT * NET, space="PSUM") as psum,
        ):
            ones = const.tile([1, P], mm_dt)
            nc.any.memset(ones[:], 1.0)
            zbias = const.tile([P, 1], mybir.dt.float32)
            nc.any.memset(zbias[:], 0.0)

            def x_piece_dma(eng, xp, c, i):
                eng.dma_start(
                    xp[:],
                    xt[c, i * XKC * P : (i + 1) * XKC * P, :].rearrange(
                        "(k p) b -> p k b", p=P
                    ),
                )

            # x pieces + bias per channel, created lazily so prefetches can be
            # emitted from inside the previous channel's compute loop.
            xtiles: dict[int, list] = {}
            btiles: dict[int, object] = {}

            def prefetch_channel(c, eng):
                xtiles[c] = [
                    xpool.tile([P, XKC, B], mm_dt, name="xp") for _ in range(XP)
                ]
                for i in range(XP):
                    x_piece_dma(eng, xtiles[c][i], c, i)
                bsb = bpool.tile([1, E], mm_dt)
                eng.dma_start(bsb[:], bias[c : c + 1, :])
                btiles[c] = bsb

            for c in range(CPC):
                if c == 0:
                    # latency-critical first channel: x piece 0 rides the fast
                    # SP-HWDGE (sync) ring ahead of the W chunks it feeds
                    xtiles[0] = [
                        xpool.tile([P, XKC, B], mm_dt, name="xp")
                        for _ in range(XP)
                    ]
                    x_piece_dma(nc.sync, xtiles[0][0], 0, 0)
                    bsb = bpool.tile([1, E], mm_dt)
                    nc.gpsimd.dma_start(bsb[:], bias[0:1, :])
                    btiles[0] = bsb
                xps = xtiles[c]
                bsb = btiles[c]

                ps = [
                    [
                        psum.tile([P, FREE], mybir.dt.float32, name="ps")
                        for _ in range(NET)
                    ]
                    for _ in range(NBT)
                ]
                # W chunk schedule (k-tiles per DMA): ramp up at kernel start
                # so the first matmuls don't wait on a full 2 MB transfer.
                chunk_kts = [1, 1, 2, 4, 4, 4] if c == 0 else [KC] * (KT // KC)
                k = 0
                for ci, ckt in enumerate(chunk_kts):
                    if c == 0 and k < XKC <= k + ckt:
                        # second x piece of channel 0, just ahead of its W
                        x_piece_dma(nc.sync, xtiles[0][1], 0, 1)
                    wsb = wpool.tile([P, KC, E], mm_dt, name="wsb")
                    nc.sync.dma_start(
                        wsb[:, :ckt, :],
                        w[c, k * P : (k + ckt) * P, :].rearrange(
                            "(k p) e -> p k e", p=P
                        ),
                    )
                    for kk in range(ckt):
                        for bt in range(NBT):
                            xp = xps[k // XKC]
                            lhsT = xp[:, k % XKC, bt * P : (bt + 1) * P]
                            for et in range(NET):
                                nc.tensor.matmul(
                                    ps[bt][et][:],
                                    lhsT,
                                    wsb[:, kk, et * FREE : (et + 1) * FREE],
                                    start=(k == 0),
                                    stop=False,
                                )
                        k += 1
                    if ci == 1 and c + 1 < CPC:
                        # early prefetch of the next channel's x + bias on the
                        # SWDGE ring, spread out during this channel's compute
                        prefetch_channel(c + 1, nc.gpsimd)
                # bias row: psum += ones(1,128).T @ bias(1,FREE)
                for bt in range(NBT):
                    for et in range(NET):
                        nc.tensor.matmul(
                            ps[bt][et][:],
                            ones[0:1, :],
                            bsb[0:1, et * FREE : (et + 1) * FREE],
                            start=False,
                            stop=True,
                        )
                # Evict with fused ReLU, split across ScalarE (activation) and
                # VectorE (max with 0) so PSUM banks free up ~2x faster.
                last = c == CPC - 1
                for bt in range(NBT):
                    ot = opool.tile([P, E], mybir.dt.float32)
                    for et in range(NET):
                        dst = ot[:, et * FREE : (et + 1) * FREE]
                        if et % 2 == 0:
                            nc.scalar.activation(
                                dst,
                                ps[bt][et][:],
                                mybir.ActivationFunctionType.Relu,
                                bias=zbias[:],
                            )
                        else:
                            nc.vector.tensor_scalar_max(dst, ps[bt][et][:], 0.0)
                        if last:
                            # tail: small per-et DMAs so the final writes
                            # start as soon as each subtile is ready
                            nc.scalar.dma_start(
                                out[
                                    bt * P : (bt + 1) * P,
                                    c,
                                    et * FREE : (et + 1) * FREE,
                                ],
                                dst,
                            )
                    if not last:
                        # one 1 MB DMA per (bt, c) on the ACT HWDGE ring,
                        # separate from the W stream
                        nc.scalar.dma_start(out[bt * P : (bt + 1) * P, c, :], ot[:])
    nc.compile()
    return nc


def _get_nc(mm_dtype: str):
    if mm_dtype not in _nc_cache:
        _nc_cache[mm_dtype] = _build(mm_dtype)
    return _nc_cache[mm_dtype]


def _run(x, W, b, mm_dtype=None, **spmd_kwargs):
    mm_dtype = mm_dtype or MM_DTYPE
    _, np_dt = _DTYPES[mm_dtype]
    nc = _get_nc(mm_dtype)

    in_maps = []
    for i in range(NCORES):
        c0, c1 = i * CPC, (i + 1) * CPC
        xt_i = np.ascontiguousarray(
            x[:, :, c0:c1].transpose(2, 1, 0).astype(np_dt)
        )
        w_i = np.ascontiguousarray(W[c0:c1].astype(np_dt))
        b_i = np.ascontiguousarray(b[c0:c1].astype(np_dt))
        in_maps.append({"xt": xt_i, "w": w_i, "bias": b_i})

    res = run_bass_kernel_spmd(nc, in_maps, core_ids=list(range(NCORES)), **spmd_kwargs)
    out = np.concatenate([r["out"] for r in res.results], axis=1)
    return out, res


def kernel(x: np.ndarray, W: np.ndarray, b: np.ndarray) -> np.ndarray:
    out, _ = _run(x, W, b)
    return out
